# revision 1
# baseline (speedup 1.0000x reference)
"""HGAT layer Trainium2 Bass kernel.

Math (per batch element b, per group pair):
  q,k,v = relu(x @ w + b) for each group
  4 masked attentions (00, 11, 01, 10), each NH=4 heads of H=32
  inner/inter = relu(attn @ wo + bo); out_g = concat(inner_g, inter_g) @ wf_g + bf_g

Device-side design (per core, 4 batch elements, data-parallel over B=32):
  - Everything is computed in "transposed" orientation (features on SBUF
    partitions): Q^T/K^T = relu(w.T @ x^T + b), V natural [k, feat].
  - scores^T[k,q] = K_h^T.T @ Q_h^T per head, row-packed 4 heads via
    tile_position row groups (contraction = 32).
  - e = exp(scores/sqrt(dk)) on ACT (PSUM->SBUF, fp16 out),
    P^T = e * mask^T on DVE (fp16 tensor_tensor, 2x mode).
  - attn_raw^T = V_chunk.T @ P^T col-packed 4 heads (tile_position col
    groups, M=32); denominators via ones[128,32] lhsT the same way --
    gives denom broadcast over each head's 32 partitions.
  - reciprocal of denom rows batched per-b on DVE, broadcast back to 128
    partitions with a small selector matmul, normalize with one TT mul.
  - wo/wf projections stay transposed; host transposes the output back.

The masks are int32 0/1; they are host-converted to fp16 (exact) and
host-transposed/duplicated so the device reads them in the exact SBUF
layout needed ([mT_c | mT_c] per 128-row chunk, giving FD=1024 DVE ops).
"""

import sys

sys.path.insert(0, "/opt/trn_rl_repo")

import numpy as np

import concourse.bacc as bacc
import concourse.tile as tile
from concourse import mybir

B, N, NH, H = 32, 512, 4, 32
IN_DIM, OUT_DIM = 128, 128
NCORES = 8
BS = B // NCORES  # batch elements per core
SQRT_DK = float(np.sqrt(H))
F32 = mybir.dt.float32
F16 = mybir.dt.float16
ADD = mybir.AluOpType.add
MAX = mybir.AluOpType.max
MULT = mybir.AluOpType.mult
EXP = mybir.ActivationFunctionType.Exp

# pair p -> (q group, k/v group); mask m{qg}{kg}; wo{qg}{kg}
PAIRS = [(0, 0), (1, 1), (0, 1), (1, 0)]
# pair -> (out group, concat row offset): inner pairs at rows 0:32, inter at 32:64
PAIR_DEST = [(0, 0), (1, 0), (0, 32), (1, 32)]


def _emit_qkv(nc, pools, W, b, g, qt, kt, vt):
    """Emit QKV projection for (b, g). Fills qt/kt [128,512] f32, vt [128,512] f16."""
    xt_t = pools["xt"].tile([128, N], F32, tag="xt", name="xt")
    nc.sync.dma_start(out=xt_t[:], in_=W["xt_ap"][b * 2 + g])

    qp = pools["sc"].tile([128, N], F32, tag="sc", name="sc")
    nc.tensor.matmul(qp[:], W["wq"][g][:], xt_t[:], start=True, stop=True)
    nc.vector.tensor_scalar(qt[:], qp[:], W["bq"][g][:], 0.0, op0=ADD, op1=MAX)

    kp = pools["sc"].tile([128, N], F32, tag="sc", name="sc")
    nc.tensor.matmul(kp[:], W["wk"][g][:], xt_t[:], start=True, stop=True)
    nc.vector.tensor_scalar(kt[:], kp[:], W["bk"][g][:], 0.0, op0=ADD, op1=MAX)

    vp = pools["sc"].tile([128, N], F32, tag="sc", name="sc")
    # full-bank bias write opens the accumulation group (orders all chains)
    nc.tensor.matmul(vp[:], W["onesrow"][:], W["bvr4"][g][:], start=True, stop=False)
    for c in range(4):
        nc.tensor.matmul(
            vp[:, 128 * c : 128 * (c + 1)],
            xt_t[:, 128 * c : 128 * (c + 1)],
            W["wv"][g][:],
            start=False,
            stop=False,
        )
    # full-bank +0 accumulate closes the group (runs after all chains)
    nc.tensor.matmul(vp[:], W["zrow32"][:], xt_t[0:1, :], start=False, stop=True)
    nc.vector.tensor_scalar_max(vt[:], vp[:], 0.0)


def _emit_attn_b(nc, pools, W, b, qt, kt, vt, cc):
    """Emit the 4 attention pairs + wo for batch element b."""
    for p, (qg, kg) in enumerate(PAIRS):
        mt_t = pools["mt"].tile([128, 4 * 1024], F16, tag="mt", name="mt")
        nc.sync.dma_start(out=mt_t[:], in_=W["mt_ap"][b * 4 + p])
        av = pools["av"].tile([128, N], F32, tag="av", name="av")
        den = pools["den"].tile([128, N], F32, tag="den", name="den")
        nc.tensor.matmul(av[:], W["zrow16"][:], mt_t[0:1, 0:N], start=True, stop=False)
        nc.tensor.matmul(den[:], W["zrow16"][:], mt_t[0:1, 0:N], start=True, stop=False)
        n_mm = 0
        for c in range(4):
            for hh in range(2):
                sc = pools["sc"].tile([128, 1024], F32, tag="sc", name="sc")
                for j in range(2):
                    h = 2 * hh + j
                    nc.tensor.matmul(
                        sc[:, 512 * j : 512 * (j + 1)],
                        kt[kg][32 * h : 32 * (h + 1), 128 * c : 128 * (c + 1)],
                        qt[qg][32 * h : 32 * (h + 1), :],
                        start=True,
                        stop=True,
                        tile_position=(32 * h, 0),
                    )
                e = pools["e"].tile([128, 1024], F16, tag="e", name="e")
                nc.scalar.activation(e[:], sc[:], EXP, scale=1.0 / SQRT_DK)
                pt = pools["e"].tile([128, 1024], F16, tag="pt", name="pt")
                nc.vector.tensor_tensor(
                    pt[:], e[:], mt_t[:, 1024 * c : 1024 * (c + 1)], op=MULT
                )
                for j in range(2):
                    h = 2 * hh + j
                    nc.tensor.matmul(
                        av[32 * h : 32 * (h + 1), :],
                        vt[kg][:, 128 * c + 32 * h : 128 * c + 32 * (h + 1)],
                        pt[:, 512 * j : 512 * (j + 1)],
                        start=False,
                        stop=False,
                        tile_position=(0, 32 * h),
                    )
                    nc.tensor.matmul(
                        den[32 * h : 32 * (h + 1), :],
                        W["ones32"][:],
                        pt[:, 512 * j : 512 * (j + 1)],
                        start=False,
                        stop=False,
                        tile_position=(0, 32 * h),
                    )
        nc.tensor.matmul(av[:], W["zrow16"][:], mt_t[0:1, 0:N], start=False, stop=True)
        nc.tensor.matmul(den[:], W["zrow16"][:], mt_t[0:1, 0:N], start=False, stop=True)
        # denominators are already broadcast over each head's 32 partitions
        rcf = pools["ar"].tile([128, N], F32, tag="rcf", name="rcf")
        nc.vector.reciprocal(rcf[:], den[:])
        an = pools["an"].tile([128, N], F32, tag="an", name="an")
        nc.vector.tensor_tensor(an[:], av[:], rcf[:], op=MULT)
        g, row = PAIR_DEST[p]
        wop = pools["small"].tile([32, N], F32, tag="small", name="small")
        nc.tensor.matmul(wop[:], W["wo"][p][:], an[:], start=True, stop=True)
        nc.vector.tensor_scalar(
            cc[g][row : row + 32, :], wop[:], W["bo"][p][:], 0.0, op0=ADD, op1=MAX
        )


def _emit_out(nc, pools, W, b, g, cc):
    wfp = pools["small"].tile([128, N], F32, tag="small", name="small")
    nc.tensor.matmul(wfp[:], W["wf"][g][:], cc[g][:], start=True, stop=True)
    ot = pools["ot"].tile([128, N], F32, tag="ot", name="ot")
    nc.vector.tensor_scalar_add(ot[:], wfp[:], W["bf"][g][:])
    nc.sync.dma_start(out=W["yt_ap"][b * 2 + g], in_=ot[:])


def build_nc(n_iters: int = 1):
    """Build + compile the per-core Bass module (body repeated n_iters times)."""
    import contextlib

    nc = bacc.Bacc("TRN2", target_bir_lowering=False, debug=False)

    xt = nc.dram_tensor("xt", [BS * 2, 128, N], F32, kind="ExternalInput")
    mt = nc.dram_tensor("mt", [BS * 4, 128, 4 * 1024], F16, kind="ExternalInput")
    wqk = nc.dram_tensor("wqk", [2, 2, 128, 128], F32, kind="ExternalInput")
    wv = nc.dram_tensor("wv", [2, 128, 128], F32, kind="ExternalInput")
    bqk = nc.dram_tensor("bqk", [2, 2, 128, 1], F32, kind="ExternalInput")
    bvr4 = nc.dram_tensor("bvr4", [2, 1, 512], F32, kind="ExternalInput")
    wo = nc.dram_tensor("wo", [4, 128, 32], F32, kind="ExternalInput")
    bo = nc.dram_tensor("bo", [4, 32, 1], F32, kind="ExternalInput")
    wf = nc.dram_tensor("wf", [2, 64, 128], F32, kind="ExternalInput")
    bf = nc.dram_tensor("bf", [2, 128, 1], F32, kind="ExternalInput")
    onesrow = nc.dram_tensor("onesrow", [1, 128], F32, kind="ExternalInput")
    ones32 = nc.dram_tensor("ones32", [128, 32], F16, kind="ExternalInput")
    yt = nc.dram_tensor("yt", [BS * 2, 128, N], F32, kind="ExternalOutput")

    with tile.TileContext(nc) as tc, contextlib.ExitStack() as ctx:
        pools = {
            "consts": ctx.enter_context(tc.tile_pool(name="consts", bufs=1)),
            "xt": ctx.enter_context(tc.tile_pool(name="xt", bufs=3)),
            "persist": ctx.enter_context(tc.tile_pool(name="persist", bufs=1)),
            "mt": ctx.enter_context(tc.tile_pool(name="mt", bufs=2)),
            "e": ctx.enter_context(tc.tile_pool(name="e", bufs=3)),
            "ar": ctx.enter_context(tc.tile_pool(name="ar", bufs=5)),
            "an": ctx.enter_context(tc.tile_pool(name="an", bufs=2)),
            "ot": ctx.enter_context(tc.tile_pool(name="ot", bufs=2)),
            "sc": ctx.enter_context(tc.tile_pool(name="sc", bufs=2, space="PSUM")),
            "av": ctx.enter_context(tc.tile_pool(name="av", bufs=1, space="PSUM")),
            "den": ctx.enter_context(tc.tile_pool(name="den", bufs=1, space="PSUM")),
            "small": ctx.enter_context(tc.tile_pool(name="small", bufs=2, space="PSUM")),
        }
        cp = pools["consts"]
        W = {
            "xt_ap": xt.ap(),
            "mt_ap": mt.ap(),
            "yt_ap": yt.ap(),
            "wq": [cp.tile([128, 128], F32, tag=f"wq{g}", name=f"wq{g}") for g in range(2)],
            "wk": [cp.tile([128, 128], F32, tag=f"wk{g}", name=f"wk{g}") for g in range(2)],
            "wv": [cp.tile([128, 128], F32, tag=f"wv{g}", name=f"wv{g}") for g in range(2)],
            "bq": [cp.tile([128, 1], F32, tag=f"bq{g}", name=f"bq{g}") for g in range(2)],
            "bk": [cp.tile([128, 1], F32, tag=f"bk{g}", name=f"bk{g}") for g in range(2)],
            "bvr4": [cp.tile([1, 512], F32, tag=f"bvr4{g}", name=f"bvr4{g}") for g in range(2)],
            "zrow16": cp.tile([1, 128], F16, tag="zrow16", name="zrow16"),
            "zrow32": cp.tile([1, 128], F32, tag="zrow32", name="zrow32"),
            "wo": [cp.tile([128, 32], F32, tag=f"wo{p}", name=f"wo{p}") for p in range(4)],
            "bo": [cp.tile([32, 1], F32, tag=f"bo{p}", name=f"bo{p}") for p in range(4)],
            "wf": [cp.tile([64, 128], F32, tag=f"wf{g}", name=f"wf{g}") for g in range(2)],
            "bf": [cp.tile([128, 1], F32, tag=f"bf{g}", name=f"bf{g}") for g in range(2)],
            "onesrow": cp.tile([1, 128], F32, tag="onesrow", name="onesrow"),
            "ones32": cp.tile([128, 32], F16, tag="ones32", name="ones32"),
        }
        for g in range(2):
            nc.sync.dma_start(out=W["wq"][g][:], in_=wqk.ap()[g, 0])
            nc.sync.dma_start(out=W["wk"][g][:], in_=wqk.ap()[g, 1])
            nc.sync.dma_start(out=W["wv"][g][:], in_=wv.ap()[g])
            nc.sync.dma_start(out=W["bq"][g][:], in_=bqk.ap()[g, 0])
            nc.sync.dma_start(out=W["bk"][g][:], in_=bqk.ap()[g, 1])
            nc.sync.dma_start(out=W["bvr4"][g][:], in_=bvr4.ap()[g])
            nc.sync.dma_start(out=W["wf"][g][:], in_=wf.ap()[g])
            nc.sync.dma_start(out=W["bf"][g][:], in_=bf.ap()[g])
        for p in range(4):
            nc.sync.dma_start(out=W["wo"][p][:], in_=wo.ap()[p])
            nc.sync.dma_start(out=W["bo"][p][:], in_=bo.ap()[p])
        nc.sync.dma_start(out=W["onesrow"][:], in_=onesrow.ap())
        nc.vector.memset(W["zrow16"][:], 0.0)
        nc.vector.memset(W["zrow32"][:], 0.0)
        nc.sync.dma_start(out=W["ones32"][:], in_=ones32.ap())

        pp = pools["persist"]
        for it in range(n_iters):
            sfx = ""
            qt = [
                [pp.tile([128, N], F32, tag=f"qt{b}{g}{sfx}", name=f"qt{b}{g}{sfx}") for g in range(2)]
                for b in range(BS)
            ]
            kt = [
                [pp.tile([128, N], F32, tag=f"kt{b}{g}{sfx}", name=f"kt{b}{g}{sfx}") for g in range(2)]
                for b in range(BS)
            ]
            vt = [
                [pp.tile([128, N], F16, tag=f"vt{b}{g}{sfx}", name=f"vt{b}{g}{sfx}") for g in range(2)]
                for b in range(BS)
            ]
            cc = [
                [pp.tile([64, N], F32, tag=f"cc{b}{g}{sfx}", name=f"cc{b}{g}{sfx}") for g in range(2)]
                for b in range(BS)
            ]
            # staggered emission: QKV(b+1) interleaves with attention(b)
            for g in range(2):
                _emit_qkv(nc, pools, W, 0, g, qt[0][g], kt[0][g], vt[0][g])
            for b in range(BS):
                if b + 1 < BS:
                    for g in range(2):
                        _emit_qkv(
                            nc, pools, W, b + 1, g, qt[b + 1][g], kt[b + 1][g], vt[b + 1][g]
                        )
                _emit_attn_b(nc, pools, W, b, qt[b], kt[b], vt[b], cc[b])
                for g in range(2):
                    _emit_out(nc, pools, W, b, g, cc[b])

    nc.compile()
    return nc


def prep_weights(inp):
    """Host-side packing of the (core-replicated) weight tensors."""
    f = np.asarray
    W = {}
    W["wqk"] = np.stack(
        [
            np.stack([f(inp["wq0"]), f(inp["wk0"])]),
            np.stack([f(inp["wq1"]), f(inp["wk1"])]),
        ]
    ).astype(np.float32)
    W["wv"] = np.stack([f(inp["wv0"]), f(inp["wv1"])]).astype(np.float32)
    W["bqk"] = np.stack(
        [
            np.stack([f(inp["bq0"]).reshape(128, 1), f(inp["bk0"]).reshape(128, 1)]),
            np.stack([f(inp["bq1"]).reshape(128, 1), f(inp["bk1"]).reshape(128, 1)]),
        ]
    ).astype(np.float32)
    W["bvr4"] = np.stack(
        [np.tile(f(inp["bv0"]), 4).reshape(1, 512), np.tile(f(inp["bv1"]), 4).reshape(1, 512)]
    ).astype(np.float32)
    W["wo"] = np.stack(
        [f(inp["wo00"]), f(inp["wo11"]), f(inp["wo01"]), f(inp["wo10"])]
    ).astype(np.float32)
    W["bo"] = np.stack(
        [
            f(inp["bo00"]).reshape(32, 1),
            f(inp["bo11"]).reshape(32, 1),
            f(inp["bo01"]).reshape(32, 1),
            f(inp["bo10"]).reshape(32, 1),
        ]
    ).astype(np.float32)
    W["wf"] = np.stack([f(inp["wf0"]), f(inp["wf1"])]).astype(np.float32)
    W["bf"] = np.stack(
        [f(inp["bf0"]).reshape(128, 1), f(inp["bf1"]).reshape(128, 1)]
    ).astype(np.float32)
    W["onesrow"] = np.ones((1, 128), np.float32)
    W["ones32"] = np.ones((128, 32), np.float16)
    return W


def prep_core_inputs(inp, W):
    """Build the 8 per-core in_maps (shards batch over cores)."""
    x = [np.asarray(inp["x0"], np.float32), np.asarray(inp["x1"], np.float32)]
    masks = [
        np.asarray(inp["m00"]),
        np.asarray(inp["m11"]),
        np.asarray(inp["m01"]),
        np.asarray(inp["m10"]),
    ]
    in_maps = []
    for ci in range(NCORES):
        xt = np.empty((BS * 2, 128, N), np.float32)
        mt = np.empty((BS * 4, 128, 4 * 1024), np.float16)
        for b in range(BS):
            gb = ci * BS + b
            for g in range(2):
                xt[b * 2 + g] = x[g][gb].T
            for p in range(4):
                mT = masks[p][gb].T.astype(np.float16)  # [k, q]
                ch = mT.reshape(4, 128, N)  # chunk c = k rows 128c..
                dup = np.stack([ch, ch], axis=1)  # [4, 2, 128, N]
                mt[b * 4 + p] = dup.transpose(2, 0, 1, 3).reshape(128, 4 * 1024)
        m = {"xt": xt, "mt": mt}
        m.update(W)
        in_maps.append(m)
    return in_maps


def postprocess(results):
    """Gather per-core yt [8,128,512] -> (out0, out1) full arrays."""
    out0 = np.empty((B, N, OUT_DIM), np.float32)
    out1 = np.empty((B, N, OUT_DIM), np.float32)
    for ci in range(NCORES):
        yt = results[ci]["yt"]
        for b in range(BS):
            gb = ci * BS + b
            out0[gb] = yt[b * 2 + 0].T
            out1[gb] = yt[b * 2 + 1].T
    return out0, out1


_NC_CACHE = {}


def get_nc(n_iters: int = 1):
    if n_iters not in _NC_CACHE:
        _NC_CACHE[n_iters] = build_nc(n_iters)
    return _NC_CACHE[n_iters]


def kernel(**inputs):
    from concourse import bass_utils

    nc = get_nc(1)
    W = prep_weights(inputs)
    in_maps = prep_core_inputs(inputs, W)
    res = bass_utils.run_bass_kernel_spmd(
        nc, in_maps, core_ids=list(range(NCORES)), trace=False
    )
    return postprocess(res.results)



# revision 4
# speedup vs baseline: 1.5165x; 1.5165x over previous
"""HGAT layer Trainium2 Bass kernel.

Math (per batch element b, per group pair):
  q,k,v = relu(x @ w + b) for each group
  4 masked attentions (00, 11, 01, 10), each NH=4 heads of H=32
  inner/inter = relu(attn @ wo + bo); out_g = concat(inner_g, inter_g) @ wf_g + bf_g

Device-side design (per core, 4 batch elements, data-parallel over B=32):
  - Transposed orientation (features on SBUF partitions). QKV projections in
    fp16 (moving operand xt fp16, 1 cyc/row).
  - q/k stored fp8e4m3 in DoubleRow layout [128, 2, 512]: head h occupies
    partitions 32h..32h+16; feature f of head h lives at (partition
    32h + f%16, t = f//16). Produced by 2 matmuls per tensor with
    col-permuted/zero-padded weight copies.
  - scores^T[k,q] per (chunk c, head h) in ONE DoubleRow fp8 matmul
    (256 cycles instead of 512 fp16).
  - mask folded into PSUM: sc += 192*mask via a 192*I fp8 matmul
    (start=False accumulate), then e = Exp(sc/sqrt(dk) - 192/sqrt(dk)) on ACT
    writes masked probabilities pt directly as fp8e4m3 (masked -> exp(-30) -> 0).
  - v stored fp8e4m3 in DR layout [128, 2(t), 2(c2), 128]: V[k,f] at
    (partition k%128, t=(k//128)%2, c2=k//256). av and den are DoubleRow
    matmuls over chunk-pairs c2 (contraction 256/call): 8+8 calls of 256
    cycles per (b,pair). den via an all-ones fp8 lhsT, broadcast over each
    head's 32 partitions (as baseline).
  - reciprocal + normalize TT on DVE, wo/wf matmuls with fp16 moving operands.

Masks are host-converted to fp8e4m3 (exact for 0/1), transposed, and
duplicated pairwise so one [128,1024] inject covers a 2-head PSUM tile.
"""

import sys

sys.path.insert(0, "/opt/trn_rl_repo")

import numpy as np

import concourse.bacc as bacc
import concourse.tile as tile
from concourse import mybir

B, N, NH, H = 32, 512, 4, 32
IN_DIM, OUT_DIM = 128, 128
NCORES = 8
BS = B // NCORES  # batch elements per core
SQRT_DK = float(np.sqrt(H))
BIGC = 192.0  # mask offset; exp((s-192)/sqrt(32)) ~ 1e-15 -> fp8 0
F32 = mybir.dt.float32
F16 = mybir.dt.float16
F8 = mybir.dt.float8e4
ADD = mybir.AluOpType.add
MAX = mybir.AluOpType.max
MULT = mybir.AluOpType.mult
EXP = mybir.ActivationFunctionType.Exp
DR = mybir.MatmulPerfMode.DoubleRow

# pair p -> (q group, k/v group); mask m{qg}{kg}; wo{qg}{kg}
PAIRS = [(0, 0), (1, 1), (0, 1), (1, 0)]
# pair -> (out group, concat row offset): inner pairs at rows 0:32, inter at 32:64
PAIR_DEST = [(0, 0), (1, 0), (0, 32), (1, 32)]
# V-projection chunk c -> vp free block (t*2 + c2), t = c%2, c2 = c//2
VBLK = [0, 2, 1, 3]


def _emit_qkv(nc, pools, W, b, g, qt, kt, vt):
    """QKV for (b, g): qt/kt [128,2,512] fp8 DR layout, vt [128,2,2,128] fp8."""
    xt_t = pools["xt"].tile([128, N], F16, tag="xt", name="xt")
    nc.sync.dma_start(out=xt_t[:], in_=W["xt_ap"][b * 2 + g])

    for qk, (dst, bias) in enumerate(((qt, "bq"), (kt, "bk"))):
        for t in range(2):
            pp = pools["sc"].tile([128, N], F32, tag="sc", name="sc")
            nc.tensor.matmul(
                pp[:], W["wqk"][g][qk][t][:], xt_t[:], start=True, stop=True
            )
            nc.vector.tensor_scalar(
                dst[:, t, :], pp[:], W[bias][g][t][:], 0.0, op0=ADD, op1=MAX
            )

    vp = pools["sc"].tile([128, N], F32, tag="sc", name="sc")
    # full-bank bias write opens the accumulation group (orders all chains)
    nc.tensor.matmul(vp[:], W["onesrow"][:], W["bvr4"][g][:], start=True, stop=False)
    for c in range(4):
        blk = VBLK[c]
        nc.tensor.matmul(
            vp[:, 128 * blk : 128 * (blk + 1)],
            xt_t[:, 128 * c : 128 * (c + 1)],
            W["wv"][g][:],
            start=False,
            stop=False,
        )
    # full-bank +0 accumulate closes the group (runs after all chains)
    nc.tensor.matmul(vp[:], W["zrow16"][:], xt_t[0:1, :], start=False, stop=True)
    nc.vector.tensor_scalar_max(vt[:, :, :, :], vp[:], 0.0)


def _emit_attn_b(nc, pools, W, b, qt, kt, vt, cc):
    """Emit the 4 attention pairs + wo for batch element b."""
    for p, (qg, kg) in enumerate(PAIRS):
        mt_t = pools["mt"].tile([128, 4 * 1024], F8, tag="mt", name="mt")
        nc.sync.dma_start(out=mt_t[:], in_=W["mt_ap"][b * 4 + p])
        pt = pools["pt"].tile([128, 2, 4, 2, 512], F8, tag="pt", name="pt")
        for c in range(4):
            c2, t = c // 2, c % 2
            for hh in range(2):
                sc = pools["sc"].tile([128, 1024], F32, tag="sc", name="sc")
                for j in range(2):
                    h = 2 * hh + j
                    nc.tensor.matmul(
                        sc[:, 512 * j : 512 * (j + 1)],
                        kt[qg if False else kg][32 * h : 32 * h + 16, :, 128 * c : 128 * (c + 1)],
                        qt[qg][32 * h : 32 * h + 16, :, :],
                        start=True,
                        stop=False,
                        tile_position=(32 * h, 0),
                        perf_mode=DR,
                        skip_group_check=True,
                    )
                # mask inject: sc += 192 * [mT_c | mT_c]
                nc.tensor.matmul(
                    sc[:],
                    W["i192"][:],
                    mt_t[:, 1024 * c : 1024 * (c + 1)],
                    start=False,
                    stop=True,
                    skip_group_check=True,
                )
                nc.scalar.activation(
                    pt[:, c2, 2 * hh : 2 * hh + 2, t, :],
                    sc[:],
                    EXP,
                    scale=1.0 / SQRT_DK,
                    bias=-BIGC / SQRT_DK,
                )
        av = pools["av"].tile([128, N], F32, tag="av", name="av")
        den = pools["den"].tile([128, N], F32, tag="den", name="den")
        for h in range(4):
            for c2 in range(2):
                nc.tensor.matmul(
                    av[32 * h : 32 * (h + 1), :],
                    vt[kg][:, :, c2, 32 * h : 32 * (h + 1)],
                    pt[:, c2, h, :, :],
                    start=(c2 == 0),
                    stop=(c2 == 1),
                    tile_position=(0, 32 * h),
                    perf_mode=DR,
                    skip_group_check=True,
                )
                nc.tensor.matmul(
                    den[32 * h : 32 * (h + 1), :],
                    W["ones8"][:, :, :],
                    pt[:, c2, h, :, :],
                    start=(c2 == 0),
                    stop=(c2 == 1),
                    tile_position=(0, 32 * h),
                    perf_mode=DR,
                    skip_group_check=True,
                )
        # denominators are already broadcast over each head's 32 partitions
        rcf = pools["ar"].tile([128, N], F32, tag="rcf", name="rcf")
        nc.vector.reciprocal(rcf[:], den[:])
        an = pools["an"].tile([128, N], F16, tag="an", name="an")
        nc.vector.tensor_tensor(an[:], av[:], rcf[:], op=MULT)
        g, row = PAIR_DEST[p]
        wop = pools["small"].tile([32, N], F32, tag="small", name="small")
        nc.tensor.matmul(wop[:], W["wo"][p][:], an[:], start=True, stop=True)
        nc.vector.tensor_scalar(
            cc[g][row : row + 32, :], wop[:], W["bo"][p][:], 0.0, op0=ADD, op1=MAX
        )


def _emit_out(nc, pools, W, b, g, cc):
    wfp = pools["small"].tile([128, N], F32, tag="small", name="small")
    nc.tensor.matmul(wfp[:], W["wf"][g][:], cc[g][:], start=True, stop=True)
    ot = pools["ot"].tile([128, N], F32, tag="ot", name="ot")
    nc.vector.tensor_scalar_add(ot[:], wfp[:], W["bf"][g][:])
    nc.sync.dma_start(out=W["yt_ap"][b * 2 + g], in_=ot[:])


def build_nc(n_iters: int = 1):
    """Build + compile the per-core Bass module (body repeated n_iters times)."""
    import contextlib

    nc = bacc.Bacc("TRN2", target_bir_lowering=False, debug=False)

    xt = nc.dram_tensor("xt", [BS * 2, 128, N], F16, kind="ExternalInput")
    mt = nc.dram_tensor("mt", [BS * 4, 128, 4 * 1024], F8, kind="ExternalInput")
    wqk = nc.dram_tensor("wqk", [2, 2, 2, 128, 128], F16, kind="ExternalInput")
    wv = nc.dram_tensor("wv", [2, 128, 128], F16, kind="ExternalInput")
    bqk = nc.dram_tensor("bqk", [2, 2, 2, 128, 1], F32, kind="ExternalInput")
    bvr4 = nc.dram_tensor("bvr4", [2, 1, 512], F16, kind="ExternalInput")
    wo = nc.dram_tensor("wo", [4, 128, 32], F16, kind="ExternalInput")
    bo = nc.dram_tensor("bo", [4, 32, 1], F32, kind="ExternalInput")
    wf = nc.dram_tensor("wf", [2, 64, 128], F16, kind="ExternalInput")
    bf = nc.dram_tensor("bf", [2, 128, 1], F32, kind="ExternalInput")
    onesrow = nc.dram_tensor("onesrow", [1, 128], F16, kind="ExternalInput")
    ones8 = nc.dram_tensor("ones8", [128, 64], F8, kind="ExternalInput")
    i192 = nc.dram_tensor("i192", [128, 128], F8, kind="ExternalInput")
    yt = nc.dram_tensor("yt", [BS * 2, 128, N], F32, kind="ExternalOutput")

    with tile.TileContext(nc) as tc, contextlib.ExitStack() as ctx:
        pools = {
            "consts": ctx.enter_context(tc.tile_pool(name="consts", bufs=1)),
            "xt": ctx.enter_context(tc.tile_pool(name="xt", bufs=3)),
            "persist": ctx.enter_context(tc.tile_pool(name="persist", bufs=1)),
            "mt": ctx.enter_context(tc.tile_pool(name="mt", bufs=2)),
            "pt": ctx.enter_context(tc.tile_pool(name="pt", bufs=2)),
            "ar": ctx.enter_context(tc.tile_pool(name="ar", bufs=5)),
            "an": ctx.enter_context(tc.tile_pool(name="an", bufs=2)),
            "ot": ctx.enter_context(tc.tile_pool(name="ot", bufs=2)),
            "sc": ctx.enter_context(tc.tile_pool(name="sc", bufs=2, space="PSUM")),
            "av": ctx.enter_context(tc.tile_pool(name="av", bufs=1, space="PSUM")),
            "den": ctx.enter_context(tc.tile_pool(name="den", bufs=1, space="PSUM")),
            "small": ctx.enter_context(tc.tile_pool(name="small", bufs=2, space="PSUM")),
        }
        cp = pools["consts"]
        W = {
            "xt_ap": xt.ap(),
            "mt_ap": mt.ap(),
            "yt_ap": yt.ap(),
            "wqk": [
                [
                    [
                        cp.tile([128, 128], F16, tag=f"wqk{g}{qk}{t}", name=f"wqk{g}{qk}{t}")
                        for t in range(2)
                    ]
                    for qk in range(2)
                ]
                for g in range(2)
            ],
            "wv": [cp.tile([128, 128], F16, tag=f"wv{g}", name=f"wv{g}") for g in range(2)],
            "bq": [
                [cp.tile([128, 1], F32, tag=f"bq{g}{t}", name=f"bq{g}{t}") for t in range(2)]
                for g in range(2)
            ],
            "bk": [
                [cp.tile([128, 1], F32, tag=f"bk{g}{t}", name=f"bk{g}{t}") for t in range(2)]
                for g in range(2)
            ],
            "bvr4": [cp.tile([1, 512], F16, tag=f"bvr4{g}", name=f"bvr4{g}") for g in range(2)],
            "zrow16": cp.tile([1, 128], F16, tag="zrow16", name="zrow16"),
            "wo": [cp.tile([128, 32], F16, tag=f"wo{p}", name=f"wo{p}") for p in range(4)],
            "bo": [cp.tile([32, 1], F32, tag=f"bo{p}", name=f"bo{p}") for p in range(4)],
            "wf": [cp.tile([64, 128], F16, tag=f"wf{g}", name=f"wf{g}") for g in range(2)],
            "bf": [cp.tile([128, 1], F32, tag=f"bf{g}", name=f"bf{g}") for g in range(2)],
            "onesrow": cp.tile([1, 128], F16, tag="onesrow", name="onesrow"),
            "ones8": cp.tile([128, 2, 32], F8, tag="ones8", name="ones8"),
            "i192": cp.tile([128, 128], F8, tag="i192", name="i192"),
        }
        for g in range(2):
            for qk in range(2):
                for t in range(2):
                    nc.sync.dma_start(out=W["wqk"][g][qk][t][:], in_=wqk.ap()[g, qk, t])
                    nc.sync.dma_start(
                        out=W[("bq", "bk")[qk]][g][t][:], in_=bqk.ap()[g, qk, t]
                    )
            nc.sync.dma_start(out=W["wv"][g][:], in_=wv.ap()[g])
            nc.sync.dma_start(out=W["bvr4"][g][:], in_=bvr4.ap()[g])
            nc.sync.dma_start(out=W["wf"][g][:], in_=wf.ap()[g])
            nc.sync.dma_start(out=W["bf"][g][:], in_=bf.ap()[g])
        for p in range(4):
            nc.sync.dma_start(out=W["wo"][p][:], in_=wo.ap()[p])
            nc.sync.dma_start(out=W["bo"][p][:], in_=bo.ap()[p])
        nc.sync.dma_start(out=W["onesrow"][:], in_=onesrow.ap())
        nc.vector.memset(W["zrow16"][:], 0.0)
        nc.sync.dma_start(out=W["ones8"][:, :, :], in_=ones8.ap())
        nc.sync.dma_start(out=W["i192"][:], in_=i192.ap())

        pp = pools["persist"]
        for it in range(n_iters):
            sfx = ""
            qt = [
                [pp.tile([128, 2, N], F8, tag=f"qt{b}{g}{sfx}", name=f"qt{b}{g}{sfx}") for g in range(2)]
                for b in range(BS)
            ]
            kt = [
                [pp.tile([128, 2, N], F8, tag=f"kt{b}{g}{sfx}", name=f"kt{b}{g}{sfx}") for g in range(2)]
                for b in range(BS)
            ]
            vt = [
                [pp.tile([128, 2, 2, 128], F8, tag=f"vt{b}{g}{sfx}", name=f"vt{b}{g}{sfx}") for g in range(2)]
                for b in range(BS)
            ]
            cc = [
                [pp.tile([64, N], F16, tag=f"cc{b}{g}{sfx}", name=f"cc{b}{g}{sfx}") for g in range(2)]
                for b in range(BS)
            ]
            # staggered emission: QKV(b+1) interleaves with attention(b)
            for g in range(2):
                _emit_qkv(nc, pools, W, 0, g, qt[0][g], kt[0][g], vt[0][g])
            for b in range(BS):
                if b + 1 < BS:
                    for g in range(2):
                        _emit_qkv(
                            nc, pools, W, b + 1, g, qt[b + 1][g], kt[b + 1][g], vt[b + 1][g]
                        )
                _emit_attn_b(nc, pools, W, b, qt[b], kt[b], vt[b], cc[b])
                for g in range(2):
                    _emit_out(nc, pools, W, b, g, cc[b])

    nc.compile()
    return nc


def _f8(x):
    import ml_dtypes

    return np.asarray(x).astype(ml_dtypes.float8_e4m3fn)


def prep_weights(inp):
    """Host-side packing of the (core-replicated) weight tensors."""
    f = np.asarray
    W = {}
    # DR col-permuted q/k weights + biases: out partition 32h+f_lo holds
    # feature 32h+16t+f_lo; cols/rows 32h+16..32h+32 zeroed.
    wqk = np.zeros((2, 2, 2, 128, 128), np.float16)
    bqk = np.zeros((2, 2, 2, 128, 1), np.float32)
    for g in range(2):
        for qk, nm in enumerate(("q", "k")):
            wsrc = f(inp[f"w{nm}{g}"])  # [128 in, 128 out-features]
            bsrc = f(inp[f"b{nm}{g}"])  # [128]
            for t in range(2):
                for h in range(4):
                    for fl in range(16):
                        feat = 32 * h + 16 * t + fl
                        wqk[g, qk, t, :, 32 * h + fl] = wsrc[:, feat]
                        bqk[g, qk, t, 32 * h + fl, 0] = bsrc[feat]
    W["wqk"] = wqk
    W["bqk"] = bqk
    W["wv"] = np.stack([f(inp["wv0"]), f(inp["wv1"])]).astype(np.float16)
    W["bvr4"] = np.stack(
        [np.tile(f(inp["bv0"]), 4).reshape(1, 512), np.tile(f(inp["bv1"]), 4).reshape(1, 512)]
    ).astype(np.float16)
    W["wo"] = np.stack(
        [f(inp["wo00"]), f(inp["wo11"]), f(inp["wo01"]), f(inp["wo10"])]
    ).astype(np.float16)
    W["bo"] = np.stack(
        [
            f(inp["bo00"]).reshape(32, 1),
            f(inp["bo11"]).reshape(32, 1),
            f(inp["bo01"]).reshape(32, 1),
            f(inp["bo10"]).reshape(32, 1),
        ]
    ).astype(np.float32)
    W["wf"] = np.stack([f(inp["wf0"]), f(inp["wf1"])]).astype(np.float16)
    W["bf"] = np.stack(
        [f(inp["bf0"]).reshape(128, 1), f(inp["bf1"]).reshape(128, 1)]
    ).astype(np.float32)
    W["onesrow"] = np.ones((1, 128), np.float16)
    W["ones8"] = _f8(np.ones((128, 64)))
    W["i192"] = _f8(BIGC * np.eye(128))
    return W


def prep_core_inputs(inp, W):
    """Build the 8 per-core in_maps (shards batch over cores)."""
    import ml_dtypes

    x = [np.asarray(inp["x0"], np.float16), np.asarray(inp["x1"], np.float16)]
    masks = [
        np.asarray(inp["m00"]),
        np.asarray(inp["m11"]),
        np.asarray(inp["m01"]),
        np.asarray(inp["m10"]),
    ]
    in_maps = []
    for ci in range(NCORES):
        xt = np.empty((BS * 2, 128, N), np.float16)
        mt = np.empty((BS * 4, 128, 4 * 1024), ml_dtypes.float8_e4m3fn)
        for b in range(BS):
            gb = ci * BS + b
            for g in range(2):
                xt[b * 2 + g] = x[g][gb].T
            for p in range(4):
                mT = _f8(masks[p][gb].T)  # [k, q]
                ch = mT.reshape(4, 128, N)  # chunk c = k rows 128c..
                dup = np.stack([ch, ch], axis=1)  # [4, 2, 128, N]
                mt[b * 4 + p] = dup.transpose(2, 0, 1, 3).reshape(128, 4 * 1024)
        m = {"xt": xt, "mt": mt}
        m.update(W)
        in_maps.append(m)
    return in_maps


def postprocess(results):
    """Gather per-core yt [8,128,512] -> (out0, out1) full arrays."""
    out0 = np.empty((B, N, OUT_DIM), np.float32)
    out1 = np.empty((B, N, OUT_DIM), np.float32)
    for ci in range(NCORES):
        yt = results[ci]["yt"]
        for b in range(BS):
            gb = ci * BS + b
            out0[gb] = yt[b * 2 + 0].T
            out1[gb] = yt[b * 2 + 1].T
    return out0, out1


_NC_CACHE = {}


def get_nc(n_iters: int = 1):
    if n_iters not in _NC_CACHE:
        _NC_CACHE[n_iters] = build_nc(n_iters)
    return _NC_CACHE[n_iters]


def kernel(**inputs):
    from concourse import bass_utils

    nc = get_nc(1)
    W = prep_weights(inputs)
    in_maps = prep_core_inputs(inputs, W)
    res = bass_utils.run_bass_kernel_spmd(
        nc, in_maps, core_ids=list(range(NCORES)), trace=False
    )
    return postprocess(res.results)
